# revision 24
# baseline (speedup 1.0000x reference)
"""Causal multi-head attention (B=4, S=2048, D=1024, H=16) on 8 Trainium2 NeuronCores.

Sharding: core c handles batch c//2 and head-group c%2 (8 of 16 heads).
Each core computes its 8 heads' qkv projection, causal attention, and its
slice of the output projection; the host sums the two half-head partials
per batch.

All matmul operands are bf16 (host-prepped): x arrives pre-transposed as
xt[d, s], weights in matmul-ready layouts with the 1/sqrt(hd) softmax
scale folded into Wq. V is produced directly in natural [k, vdim] layout
by using x^T chunks as the stationary operand, so no on-chip transposes
are needed. Softmax row-sums ride along the PV matmul as 32 ones-columns
in the stationary operand; the reciprocal runs on DVE.

Self-contained: hardcodes shapes; imports concourse from the container's
trn_rl_repo. kernel(**inputs) takes full inputs, returns full output.
"""
import sys

for _p in ("/opt/trn_rl_repo", "/root/.axon_site/_ro/trn_rl_repo"):
    if _p not in sys.path:
        sys.path.append(_p)

import numpy as np

import concourse.bass as bass
import concourse.mybir as mybir
import concourse.tile as tile
from concourse import bacc

B, S, D, H = 4, 2048, 1024, 16
HD = D // H            # 64
NHL = 8                # heads per core
QB = 1024              # attention q-block
NKC = S // 128         # 16 k-chunks per sequence
dt = mybir.dt
AF = mybir.ActivationFunctionType
P = 128


def build_nc(repeat=1):
    nc = bacc.Bacc("TRN2", target_bir_lowering=False, debug=False)

    # xt[p, dc, s] = x[s, dc*128 + p]  (pre-transposed, bf16)
    xt = nc.dram_tensor("xt", [P, 8, S], dt.bfloat16, kind="ExternalInput")
    # wqk[p, ch, dc, j]: ch 0-3 q-pairs, 4-7 k-pairs; j = hd of head pair
    wqk = nc.dram_tensor("wqk", [P, 8, 8, P], dt.bfloat16, kind="ExternalInput")
    # wv[p, dc, h*64+j] = Wv[dc*128+p, (hg+h)*64+j]
    wv = nc.dram_tensor("wv", [P, 8, 512], dt.bfloat16, kind="ExternalInput")
    # wpj[j, pc, :]: rows = vdim of head pair pc
    wpj = nc.dram_tensor("wpj", [P, 4, D], dt.bfloat16, kind="ExternalInput")
    out = nc.dram_tensor("out", [S, D], dt.float32, kind="ExternalOutput")

    from contextlib import ExitStack
    with tile.TileContext(nc) as tc, ExitStack() as _rep:
        if repeat > 1:
            _rep.enter_context(tc.For_i(0, repeat, 1))
        with tc.tile_pool(name="persist", bufs=1) as pp:

            xT = pp.tile([P, 8, S], dt.bfloat16, tag="xT")
            wqks = pp.tile([P, 8, 8, P], dt.bfloat16, tag="wqks")
            wvs = pp.tile([P, 8, 512], dt.bfloat16, tag="wvs")
            wpjs = pp.tile([P, 4, D], dt.bfloat16, tag="wpjs")

            QT = pp.tile([P, 4, S], dt.bfloat16, tag="QT")  # [hd pair, pair, s]
            KT = pp.tile([P, 4, S], dt.bfloat16, tag="KT")
            V2 = pp.tile([P, NHL, NKC, 96], dt.bfloat16, tag="V2")  # [k, h, kc, 64 V | 32 ones]
            nc.gpsimd.memset(V2[:, :, :, 64:96], 1.0)
            yT = pp.tile([P, 4, S], dt.bfloat16, tag="yT")  # [vdim pair, pair, s]

            # stage inputs in first-use order, finely split so the first
            # matmuls start as soon as possible
            nc.sync.dma_start(wqks[:, 0], wqk[:, 0])
            for dcp in range(4):
                nc.sync.dma_start(xT[:, 2 * dcp:2 * dcp + 2, 0:1024],
                                  xt[:, 2 * dcp:2 * dcp + 2, 0:1024])
            nc.sync.dma_start(wqks[:, 4], wqk[:, 4])
            nc.sync.dma_start(xT[:, :, 1024:2048], xt[:, :, 1024:2048])
            for p in range(1, 4):
                nc.sync.dma_start(wqks[:, p], wqk[:, p])
                nc.sync.dma_start(wqks[:, p + 4], wqk[:, p + 4])
            nc.sync.dma_start(wvs[:], wv[:])
            nc.sync.dma_start(wpjs[:], wpj[:])

            # ---- fused QKV production + attention + interleaved projection ----
            # PSUM: psS 2x[P,1024] (4 banks, shared by qkv-production and
            # scores) + psV 2x[P,1024] (4 banks, shared by pv and proj)
            with tc.tile_pool(name="ta", bufs=2) as ta, \
                 tc.tile_pool(name="tpt", bufs=4) as tpt, \
                 tc.tile_pool(name="tp", bufs=4) as tp, \
                 tc.tile_pool(name="psS", bufs=2, space="PSUM") as psS, \
                 tc.tile_pool(name="psV", bufs=2, space="PSUM") as psV:

                def pairQK_half(p, hf):
                    """Q and K for pair p, q-cols hf*1024..+1024."""
                    for ch in (p, p + 4):
                        psq = psS.tile([P, QB], dt.float32, tag="sc")
                        for sb2 in range(2):
                            sb = hf * 2 + sb2
                            for dc in range(8):
                                nc.tensor.matmul(psq[:, sb2 * 512:(sb2 + 1) * 512],
                                                 wqks[:, ch, dc, :],
                                                 xT[:, dc, sb * 512:(sb + 1) * 512],
                                                 start=(dc == 0), stop=(dc == 7))
                        dst = QT if ch < 4 else KT
                        nc.vector.tensor_copy(
                            dst[:, ch % 4, hf * QB:(hf + 1) * QB], psq[:])

                def V_half(hf):
                    """V for all 8 heads, k-chunks hf*8..+8 (2 s-chunks/tile)."""
                    for scp in range(4):
                        psv = psS.tile([P, QB], dt.float32, tag="sc")
                        for sc2 in range(2):
                            sc = hf * 8 + scp * 2 + sc2
                            for dc in range(8):
                                nc.tensor.matmul(psv[:, sc2 * 512:(sc2 + 1) * 512],
                                                 xT[:, dc, sc * P:(sc + 1) * P],
                                                 wvs[:, dc, :],
                                                 start=(dc == 0), stop=(dc == 7))
                        nc.vector.tensor_copy(
                            V2[:, :, hf * 8 + scp * 2:hf * 8 + scp * 2 + 2, 0:64],
                            psv[:].rearrange("p (a b c) -> p b a c", a=2, b=8))

                def attn_block(h, qb):
                    pr = h // 2
                    half = slice(0, 64) if h % 2 == 0 else slice(64, P)
                    nkc = (qb + 1) * 8
                    pv_ps = psV.tile([P, QB], dt.float32, tag="pv")
                    pend = []  # [(kc, pT tile, qlo)] — PV trails scores by 2

                    def emit_pv(kc, pT_t, qlo):
                        q0 = qlo
                        while q0 < QB:
                            q1 = min((q0 // 512 + 1) * 512, QB)  # stay within one PSUM bank
                            nc.tensor.matmul(pv_ps[0:96, q0:q1],
                                             V2[:, h, kc, :], pT_t[:, q0:q1],
                                             start=(kc == 0), stop=(kc == nkc - 1),
                                             skip_group_check=True)
                            q0 = q1

                    for kc in range(nkc):
                        qlo = max(0, kc * P - qb * QB)
                        sc_ps = psS.tile([P, QB], dt.float32, tag="sc")
                        q0 = qlo
                        while q0 < QB:
                            q1 = min((q0 // 512 + 1) * 512, QB)  # stay within one PSUM bank
                            nc.tensor.matmul(sc_ps[:, q0:q1],
                                             KT[half, pr, kc * P:(kc + 1) * P],
                                             QT[half, pr, qb * QB + q0:qb * QB + q1],
                                             start=True, stop=True)
                            q0 = q1
                        pT_t = tpt.tile([P, QB], dt.bfloat16, tag="pT")
                        nc.scalar.activation(pT_t[:, qlo:QB], sc_ps[:, qlo:QB], AF.Exp)
                        if kc * P >= qb * QB:  # diagonal chunk: zero p on k>q corner
                            nc.gpsimd.affine_select(
                                out=pT_t[:, qlo:qlo + P], in_=pT_t[:, qlo:qlo + P],
                                compare_op=mybir.AluOpType.is_ge, fill=0.0,
                                base=0, pattern=[[1, P]], channel_multiplier=-1)
                        pend.append((kc, pT_t, qlo))
                        if len(pend) > 2:
                            emit_pv(*pend.pop(0))
                    for e in pend:
                        emit_pv(*e)

                    # normalization: y = pv * (1/sums); sums dup on rows 64:96
                    trc = ta.tile([P, QB], dt.float32, tag="trc")
                    nc.vector.reciprocal(trc[64:96, :], pv_ps[64:96, :])
                    rsh = ta.tile([64, QB], dt.float32, tag="rsh")
                    nc.sync.dma_start(rsh[0:32, :], trc[64:96, :])
                    nc.sync.dma_start(rsh[32:64, :], trc[64:96, :])
                    if h % 2 == 0:
                        nc.vector.tensor_tensor(yT[0:64, pr, qb * QB:(qb + 1) * QB],
                                                pv_ps[0:64, :], rsh[:],
                                                mybir.AluOpType.mult)
                    else:
                        ytmp = ta.tile([64, QB], dt.bfloat16, tag="ytmp")
                        nc.vector.tensor_tensor(ytmp[:], pv_ps[0:64, :], rsh[:],
                                                mybir.AluOpType.mult)
                        nc.sync.dma_start(yT[64:P, pr, qb * QB:(qb + 1) * QB], ytmp[:])

                def proj_chunk(sc):
                    # psum shared with psV pool (same tag/shape -> same bufs ring)
                    pps = psV.tile([P, QB], dt.float32, tag="pv")
                    for oc in range(2):
                        for pc in range(4):
                            nc.tensor.matmul(pps[:, oc * 512:(oc + 1) * 512],
                                             yT[:, pc, sc * P:(sc + 1) * P],
                                             wpjs[:, pc, oc * 512:(oc + 1) * 512],
                                             start=(pc == 0), stop=(pc == 3))
                    so = tp.tile([P, D], dt.float32, tag="so")
                    # alternate copy engine so neither ACT nor DVE throttles PE
                    if sc % 2 == 0:
                        nc.vector.tensor_copy(so[:], pps[:])
                    else:
                        nc.scalar.copy(so[:], pps[:])
                    nc.sync.dma_start(out[sc * P:(sc + 1) * P, :], so[:])

                # solid QKV phase (dense PE stream keeps the p-state ramp warm)
                for p in range(4):
                    pairQK_half(p, 0)
                    pairQK_half(p, 1)
                V_half(0)
                V_half(1)
                # qb0+qb1 adjacent: the long qb1 block covers the qb0 norm
                # chain, and the final block's chain drains under proj work
                for h in range(NHL):
                    attn_block(h, 0)
                    attn_block(h, 1)
                for sc in range(16):
                    proj_chunk(sc)

    nc.compile()
    return nc


def prepare_inputs(x, Wqkv, Wproj):
    """Pack per-core bf16 inputs. Core c: batch c//2, heads (c%2)*8 .. +8."""
    from ml_dtypes import bfloat16
    x = np.asarray(x, dtype=np.float32)
    Wqkv = np.asarray(Wqkv, dtype=np.float32)
    Wproj = np.asarray(Wproj, dtype=np.float32)
    scale = 1.0 / np.sqrt(HD)
    # Wqkv rows d = dc*128 + p
    Wq = (Wqkv[:, :D] * scale).reshape(8, P, H, HD)  # [dc, p, head, hd]
    Wk = Wqkv[:, D:2 * D].reshape(8, P, H, HD)
    Wv_ = Wqkv[:, 2 * D:].reshape(8, P, H, HD)
    in_maps = []
    for c in range(8):
        b, g = c // 2, c % 2
        hg = g * NHL
        wqk = np.empty((P, 8, 8, P), dtype=np.float32)
        for ch in range(4):
            wqk[:, ch, :, 0:64] = Wq[:, :, hg + 2 * ch, :].transpose(1, 0, 2)
            wqk[:, ch, :, 64:P] = Wq[:, :, hg + 2 * ch + 1, :].transpose(1, 0, 2)
            wqk[:, ch + 4, :, 0:64] = Wk[:, :, hg + 2 * ch, :].transpose(1, 0, 2)
            wqk[:, ch + 4, :, 64:P] = Wk[:, :, hg + 2 * ch + 1, :].transpose(1, 0, 2)
        # wv[p, dc, h*64+j] = Wv[dc*128+p, (hg+h)*64+j]
        wv = Wv_[:, :, hg:hg + NHL, :].reshape(8, P, NHL * HD).transpose(1, 0, 2)
        wpj = np.empty((P, 4, D), dtype=np.float32)
        for pc in range(4):
            wpj[0:64, pc, :] = Wproj[HD * (hg + 2 * pc):HD * (hg + 2 * pc) + HD, :]
            wpj[64:P, pc, :] = Wproj[HD * (hg + 2 * pc + 1):HD * (hg + 2 * pc + 1) + HD, :]
        # xt[p, dc, s] = x[b, s, dc*128+p]
        xt = np.ascontiguousarray(x[b].T.reshape(8, P, S).transpose(1, 0, 2))
        in_maps.append({
            "xt": xt.astype(bfloat16),
            "wqk": np.ascontiguousarray(wqk).astype(bfloat16),
            "wv": np.ascontiguousarray(wv).astype(bfloat16),
            "wpj": wpj.astype(bfloat16),
        })
    return in_maps


def combine_outputs(results):
    out = np.empty((B, S, D), dtype=np.float32)
    for b in range(B):
        out[b] = results[2 * b]["out"] + results[2 * b + 1]["out"]
    return out


_NC_CACHE = None


def get_nc():
    global _NC_CACHE
    if _NC_CACHE is None:
        _NC_CACHE = build_nc()
    return _NC_CACHE


def kernel(x, Wqkv, Wproj):
    from concourse.bass_utils import run_bass_kernel_spmd
    nc = get_nc()
    in_maps = prepare_inputs(x, Wqkv, Wproj)
    res = run_bass_kernel_spmd(nc, in_maps, core_ids=list(range(8)))
    return combine_outputs(res.results)


if __name__ == "__main__":
    rng = np.random.default_rng(0)
    x = rng.standard_normal((B, S, D), dtype=np.float32)
    Wqkv = (rng.standard_normal((D, 3 * D), dtype=np.float32) / np.sqrt(D)).astype(np.float32)
    Wproj = (rng.standard_normal((D, D), dtype=np.float32) / np.sqrt(D)).astype(np.float32)
    y = kernel(x, Wqkv, Wproj)
    print("ok", y.shape, float(np.abs(y).max()))


# revision 26
# speedup vs baseline: 1.0576x; 1.0576x over previous
"""Causal multi-head attention (B=4, S=2048, D=1024, H=16) on 8 Trainium2 NeuronCores.

Sharding: core c handles batch c//2 and head-group c%2 (8 of 16 heads).
Each core computes its 8 heads' qkv projection, causal attention, and its
slice of the output projection; the host sums the two half-head partials
per batch.

All matmul operands are bf16 (host-prepped): x arrives pre-transposed as
xt[d, s], weights in matmul-ready layouts with the 1/sqrt(hd) softmax
scale folded into Wq. V is produced directly in natural [k, vdim] layout
by using x^T chunks as the stationary operand, so no on-chip transposes
are needed. Softmax row-sums ride along the PV matmul as 32 ones-columns
in the stationary operand; the reciprocal runs on DVE.

Self-contained: hardcodes shapes; imports concourse from the container's
trn_rl_repo. kernel(**inputs) takes full inputs, returns full output.
"""
import sys

for _p in ("/opt/trn_rl_repo", "/root/.axon_site/_ro/trn_rl_repo"):
    if _p not in sys.path:
        sys.path.append(_p)

import numpy as np

import concourse.bass as bass
import concourse.mybir as mybir
import concourse.tile as tile
from concourse import bacc

B, S, D, H = 4, 2048, 1024, 16
HD = D // H            # 64
NHL = 8                # heads per core
QB = 1024              # attention q-block
NKC = S // 128         # 16 k-chunks per sequence
dt = mybir.dt
AF = mybir.ActivationFunctionType
P = 128


RECIP_MODE = "dve"  # 'dve' | 'copy' (timing A/B only, wrong values) | 'lnexp'


def build_nc(repeat=1):
    nc = bacc.Bacc("TRN2", target_bir_lowering=False, debug=False)

    # xt[p, dc, s] = x[s, dc*128 + p]  (pre-transposed, bf16)
    xt = nc.dram_tensor("xt", [P, 8, S], dt.bfloat16, kind="ExternalInput")
    # wqk[p, ch, dc, j]: ch 0-3 q-pairs, 4-7 k-pairs; j = hd of head pair
    wqk = nc.dram_tensor("wqk", [P, 8, 8, P], dt.bfloat16, kind="ExternalInput")
    # wv[p, dc, h*64+j] = Wv[dc*128+p, (hg+h)*64+j]
    wv = nc.dram_tensor("wv", [P, 8, 512], dt.bfloat16, kind="ExternalInput")
    # wpj[j, pc, :]: rows = vdim of head pair pc
    wpj = nc.dram_tensor("wpj", [P, 4, D], dt.bfloat16, kind="ExternalInput")
    out = nc.dram_tensor("out", [S, D], dt.float32, kind="ExternalOutput")

    from contextlib import ExitStack
    with tile.TileContext(nc) as tc, ExitStack() as _rep:
        if repeat > 1:
            _rep.enter_context(tc.For_i(0, repeat, 1))
        with tc.tile_pool(name="persist", bufs=1) as pp:

            xT = pp.tile([P, 8, S], dt.bfloat16, tag="xT")
            wqks = pp.tile([P, 8, 8, P], dt.bfloat16, tag="wqks")
            wvs = pp.tile([P, 8, 512], dt.bfloat16, tag="wvs")
            wpjs = pp.tile([P, 4, D], dt.bfloat16, tag="wpjs")

            QT = pp.tile([P, 4, S], dt.bfloat16, tag="QT")  # [hd pair, pair, s]
            KT = pp.tile([P, 4, S], dt.bfloat16, tag="KT")
            V2 = pp.tile([P, NHL, NKC, 96], dt.bfloat16, tag="V2")  # [k, h, kc, 64 V | 32 ones]
            nc.gpsimd.memset(V2[:, :, :, 64:96], 1.0)
            yT = pp.tile([P, 4, S], dt.bfloat16, tag="yT")  # [vdim pair, pair, s]

            # stage inputs in first-use order, finely split so the first
            # matmuls start as soon as possible
            nc.sync.dma_start(wqks[:, 0], wqk[:, 0])
            for dcp in range(4):
                nc.sync.dma_start(xT[:, 2 * dcp:2 * dcp + 2, 0:1024],
                                  xt[:, 2 * dcp:2 * dcp + 2, 0:1024])
            nc.sync.dma_start(wqks[:, 4], wqk[:, 4])
            nc.sync.dma_start(xT[:, :, 1024:2048], xt[:, :, 1024:2048])
            for p in range(1, 4):
                nc.sync.dma_start(wqks[:, p], wqk[:, p])
                nc.sync.dma_start(wqks[:, p + 4], wqk[:, p + 4])
            nc.sync.dma_start(wvs[:], wv[:])
            nc.sync.dma_start(wpjs[:], wpj[:])

            # ---- fused QKV production + attention + interleaved projection ----
            # PSUM: psS 2x[P,1024] (4 banks, shared by qkv-production and
            # scores) + psV 2x[P,1024] (4 banks, shared by pv and proj)
            with tc.tile_pool(name="ta", bufs=2) as ta, \
                 tc.tile_pool(name="tpt", bufs=4) as tpt, \
                 tc.tile_pool(name="tp", bufs=4) as tp, \
                 tc.tile_pool(name="psS", bufs=2, space="PSUM") as psS, \
                 tc.tile_pool(name="psV", bufs=2, space="PSUM") as psV:

                def pairQK_half(p, hf):
                    """Q and K for pair p, q-cols hf*1024..+1024."""
                    for ch in (p, p + 4):
                        psq = psS.tile([P, QB], dt.float32, tag="sc")
                        for sb2 in range(2):
                            sb = hf * 2 + sb2
                            for dc in range(8):
                                nc.tensor.matmul(psq[:, sb2 * 512:(sb2 + 1) * 512],
                                                 wqks[:, ch, dc, :],
                                                 xT[:, dc, sb * 512:(sb + 1) * 512],
                                                 start=(dc == 0), stop=(dc == 7))
                        dst = QT if ch < 4 else KT
                        nc.vector.tensor_copy(
                            dst[:, ch % 4, hf * QB:(hf + 1) * QB], psq[:])

                def V_half(hf):
                    """V for all 8 heads, k-chunks hf*8..+8 (2 s-chunks/tile)."""
                    for scp in range(4):
                        psv = psS.tile([P, QB], dt.float32, tag="sc")
                        for sc2 in range(2):
                            sc = hf * 8 + scp * 2 + sc2
                            for dc in range(8):
                                nc.tensor.matmul(psv[:, sc2 * 512:(sc2 + 1) * 512],
                                                 xT[:, dc, sc * P:(sc + 1) * P],
                                                 wvs[:, dc, :],
                                                 start=(dc == 0), stop=(dc == 7))
                        nc.vector.tensor_copy(
                            V2[:, :, hf * 8 + scp * 2:hf * 8 + scp * 2 + 2, 0:64],
                            psv[:].rearrange("p (a b c) -> p b a c", a=2, b=8))

                def attn_block(h, qb):
                    pr = h // 2
                    half = slice(0, 64) if h % 2 == 0 else slice(64, P)
                    nkc = (qb + 1) * 8
                    pv_ps = psV.tile([P, QB], dt.float32, tag="pv")
                    pend = []  # [(kc, pT tile, qlo)] — PV trails scores by 2

                    def emit_pv(kc, pT_t, qlo):
                        q0 = qlo
                        while q0 < QB:
                            q1 = min((q0 // 512 + 1) * 512, QB)  # stay within one PSUM bank
                            nc.tensor.matmul(pv_ps[0:96, q0:q1],
                                             V2[:, h, kc, :], pT_t[:, q0:q1],
                                             start=(kc == 0), stop=(kc == nkc - 1),
                                             skip_group_check=True)
                            q0 = q1

                    for kc in range(nkc):
                        qlo = max(0, kc * P - qb * QB)
                        sc_ps = psS.tile([P, QB], dt.float32, tag="sc")
                        q0 = qlo
                        while q0 < QB:
                            q1 = min((q0 // 512 + 1) * 512, QB)  # stay within one PSUM bank
                            nc.tensor.matmul(sc_ps[:, q0:q1],
                                             KT[half, pr, kc * P:(kc + 1) * P],
                                             QT[half, pr, qb * QB + q0:qb * QB + q1],
                                             start=True, stop=True)
                            q0 = q1
                        pT_t = tpt.tile([P, QB], dt.bfloat16, tag="pT")
                        nc.scalar.activation(pT_t[:, qlo:QB], sc_ps[:, qlo:QB], AF.Exp)
                        if kc * P >= qb * QB:  # diagonal chunk: zero p on k>q corner
                            nc.gpsimd.affine_select(
                                out=pT_t[:, qlo:qlo + P], in_=pT_t[:, qlo:qlo + P],
                                compare_op=mybir.AluOpType.is_ge, fill=0.0,
                                base=0, pattern=[[1, P]], channel_multiplier=-1)
                        pend.append((kc, pT_t, qlo))
                        if len(pend) > 2:
                            emit_pv(*pend.pop(0))
                    for e in pend:
                        emit_pv(*e)

                    # normalization: y = pv * (1/sums); sums dup on rows 64:96
                    trc = ta.tile([P, QB], dt.float32, tag="trc")
                    if RECIP_MODE == "dve":
                        nc.vector.reciprocal(trc[64:96, :], pv_ps[64:96, :])
                    elif RECIP_MODE == "copy":
                        nc.vector.tensor_copy(trc[64:96, :], pv_ps[64:96, :])
                    else:  # lnexp on ACT
                        tln = ta.tile([P, QB], dt.float32, tag="tln")
                        nc.scalar.activation(tln[64:96, :], pv_ps[64:96, :], AF.Ln)
                        nc.scalar.activation(trc[64:96, :], tln[64:96, :], AF.Exp,
                                             scale=-1.0)
                    rsh = ta.tile([64, QB], dt.float32, tag="rsh")
                    nc.sync.dma_start(rsh[0:32, :], trc[64:96, :])
                    nc.sync.dma_start(rsh[32:64, :], trc[64:96, :])
                    if h % 2 == 0:
                        nc.vector.tensor_tensor(yT[0:64, pr, qb * QB:(qb + 1) * QB],
                                                pv_ps[0:64, :], rsh[:],
                                                mybir.AluOpType.mult)
                    else:
                        ytmp = ta.tile([64, QB], dt.bfloat16, tag="ytmp")
                        nc.vector.tensor_tensor(ytmp[:], pv_ps[0:64, :], rsh[:],
                                                mybir.AluOpType.mult)
                        nc.sync.dma_start(yT[64:P, pr, qb * QB:(qb + 1) * QB], ytmp[:])

                def proj_chunk(sc):
                    # psum shared with psV pool (same tag/shape -> same bufs ring)
                    pps = psV.tile([P, QB], dt.float32, tag="pv")
                    for oc in range(2):
                        for pc in range(4):
                            nc.tensor.matmul(pps[:, oc * 512:(oc + 1) * 512],
                                             yT[:, pc, sc * P:(sc + 1) * P],
                                             wpjs[:, pc, oc * 512:(oc + 1) * 512],
                                             start=(pc == 0), stop=(pc == 3))
                    so = tp.tile([P, D], dt.float32, tag="so")
                    # alternate copy engine so neither ACT nor DVE throttles PE
                    if sc % 2 == 0:
                        nc.vector.tensor_copy(so[:], pps[:])
                    else:
                        nc.scalar.copy(so[:], pps[:])
                    nc.sync.dma_start(out[sc * P:(sc + 1) * P, :], so[:])

                # solid QKV phase (dense PE stream keeps the p-state ramp warm)
                for p in range(4):
                    pairQK_half(p, 0)
                    pairQK_half(p, 1)
                V_half(0)
                V_half(1)
                # qb0+qb1 adjacent: the long qb1 block covers the qb0 norm
                # chain, and the final block's chain drains under proj work
                for h in range(NHL):
                    attn_block(h, 0)
                    attn_block(h, 1)
                for sc in range(16):
                    proj_chunk(sc)

    nc.compile()
    return nc


def prepare_inputs(x, Wqkv, Wproj):
    """Pack per-core bf16 inputs. Core c: batch c//2, heads (c%2)*8 .. +8."""
    from ml_dtypes import bfloat16
    x = np.asarray(x, dtype=np.float32)
    Wqkv = np.asarray(Wqkv, dtype=np.float32)
    Wproj = np.asarray(Wproj, dtype=np.float32)
    scale = 1.0 / np.sqrt(HD)
    # Wqkv rows d = dc*128 + p
    Wq = (Wqkv[:, :D] * scale).reshape(8, P, H, HD)  # [dc, p, head, hd]
    Wk = Wqkv[:, D:2 * D].reshape(8, P, H, HD)
    Wv_ = Wqkv[:, 2 * D:].reshape(8, P, H, HD)
    in_maps = []
    for c in range(8):
        b, g = c // 2, c % 2
        hg = g * NHL
        wqk = np.empty((P, 8, 8, P), dtype=np.float32)
        for ch in range(4):
            wqk[:, ch, :, 0:64] = Wq[:, :, hg + 2 * ch, :].transpose(1, 0, 2)
            wqk[:, ch, :, 64:P] = Wq[:, :, hg + 2 * ch + 1, :].transpose(1, 0, 2)
            wqk[:, ch + 4, :, 0:64] = Wk[:, :, hg + 2 * ch, :].transpose(1, 0, 2)
            wqk[:, ch + 4, :, 64:P] = Wk[:, :, hg + 2 * ch + 1, :].transpose(1, 0, 2)
        # wv[p, dc, h*64+j] = Wv[dc*128+p, (hg+h)*64+j]
        wv = Wv_[:, :, hg:hg + NHL, :].reshape(8, P, NHL * HD).transpose(1, 0, 2)
        wpj = np.empty((P, 4, D), dtype=np.float32)
        for pc in range(4):
            wpj[0:64, pc, :] = Wproj[HD * (hg + 2 * pc):HD * (hg + 2 * pc) + HD, :]
            wpj[64:P, pc, :] = Wproj[HD * (hg + 2 * pc + 1):HD * (hg + 2 * pc + 1) + HD, :]
        # xt[p, dc, s] = x[b, s, dc*128+p]
        xt = np.ascontiguousarray(x[b].T.reshape(8, P, S).transpose(1, 0, 2))
        in_maps.append({
            "xt": xt.astype(bfloat16),
            "wqk": np.ascontiguousarray(wqk).astype(bfloat16),
            "wv": np.ascontiguousarray(wv).astype(bfloat16),
            "wpj": wpj.astype(bfloat16),
        })
    return in_maps


def combine_outputs(results):
    out = np.empty((B, S, D), dtype=np.float32)
    for b in range(B):
        out[b] = results[2 * b]["out"] + results[2 * b + 1]["out"]
    return out


_NC_CACHE = None


def get_nc():
    global _NC_CACHE
    if _NC_CACHE is None:
        _NC_CACHE = build_nc()
    return _NC_CACHE


def kernel(x, Wqkv, Wproj):
    from concourse.bass_utils import run_bass_kernel_spmd
    nc = get_nc()
    in_maps = prepare_inputs(x, Wqkv, Wproj)
    res = run_bass_kernel_spmd(nc, in_maps, core_ids=list(range(8)))
    return combine_outputs(res.results)


if __name__ == "__main__":
    rng = np.random.default_rng(0)
    x = rng.standard_normal((B, S, D), dtype=np.float32)
    Wqkv = (rng.standard_normal((D, 3 * D), dtype=np.float32) / np.sqrt(D)).astype(np.float32)
    Wproj = (rng.standard_normal((D, D), dtype=np.float32) / np.sqrt(D)).astype(np.float32)
    y = kernel(x, Wqkv, Wproj)
    print("ok", y.shape, float(np.abs(y).max()))


# revision 37
# speedup vs baseline: 1.1669x; 1.1033x over previous
"""Causal multi-head attention (B=4, S=2048, D=1024, H=16) on 8 Trainium2 NeuronCores.

Sharding: core c handles batch c//2 and head-group c%2 (8 of 16 heads).
Each core computes its 8 heads' qkv projection, causal attention, and its
slice of the output projection; the host sums the two half-head partials
per batch.

All matmul operands are bf16 (host-prepped): x arrives pre-transposed as
xt[d, s], weights in matmul-ready layouts with the 1/sqrt(hd) softmax
scale folded into Wq. V is produced directly in natural [k, vdim] layout
by using x^T chunks as the stationary operand, so no on-chip transposes
are needed. Softmax row-sums ride along the PV matmul as 32 ones-columns
in the stationary operand; the reciprocal runs on DVE.

Self-contained: hardcodes shapes; imports concourse from the container's
trn_rl_repo. kernel(**inputs) takes full inputs, returns full output.
"""
import sys

for _p in ("/opt/trn_rl_repo", "/root/.axon_site/_ro/trn_rl_repo"):
    if _p not in sys.path:
        sys.path.append(_p)

import numpy as np

import concourse.bass as bass
import concourse.mybir as mybir
import concourse.tile as tile
from concourse import bacc

B, S, D, H = 4, 2048, 1024, 16
HD = D // H            # 64
NHL = 8                # heads per core
QB = 1024              # attention q-block
NKC = S // 128         # 16 k-chunks per sequence
dt = mybir.dt
AF = mybir.ActivationFunctionType
P = 128


RECIP_MODE = "dve"  # 'dve' | 'copy' (timing A/B only, wrong values) | 'lnexp'
MASK_MODE = "pool"  # 'pool' (gpsimd zero-fill on pT) | 'dve' (-1e30 add on scores)
ORDER = "paired"    # 'qbmajor' (proj interleaved into qb1) | 'paired'


def build_nc(repeat=1):
    nc = bacc.Bacc("TRN2", target_bir_lowering=False, debug=False)

    # xt[p, dc, s] = x[s, dc*128 + p]  (pre-transposed, bf16)
    xt = nc.dram_tensor("xt", [P, 8, S], dt.bfloat16, kind="ExternalInput")
    # wqk[p, ch, dc, j]: ch 0-3 q-pairs, 4-7 k-pairs; j = hd of head pair
    wqk = nc.dram_tensor("wqk", [P, 8, 8, P], dt.bfloat16, kind="ExternalInput")
    # wv[p, dc, h*64+j] = Wv[dc*128+p, (hg+h)*64+j]
    wv = nc.dram_tensor("wv", [P, 8, 512], dt.bfloat16, kind="ExternalInput")
    # wpj[j, pc, :]: rows = vdim of head pair pc
    wpj = nc.dram_tensor("wpj", [P, 4, D], dt.bfloat16, kind="ExternalInput")
    out = nc.dram_tensor("out", [S, D], dt.float32, kind="ExternalOutput")

    from contextlib import ExitStack
    with tile.TileContext(nc) as tc, ExitStack() as _rep:
        if repeat > 1:
            _rep.enter_context(tc.For_i(0, repeat, 1))
        with tc.tile_pool(name="persist", bufs=1) as pp:

            if MASK_MODE == "dve":
                # causal mask tile: 0 where f>=p else -1e30
                maskT = pp.tile([P, P], dt.float32, tag="maskT")
                nc.gpsimd.memset(maskT[:], 0.0)
                nc.gpsimd.affine_select(
                    out=maskT[:], in_=maskT[:],
                    compare_op=mybir.AluOpType.is_ge, fill=-1e30,
                    base=0, pattern=[[1, P]], channel_multiplier=-1)

            xT = pp.tile([P, 8, S], dt.bfloat16, tag="xT")
            wqks = pp.tile([P, 8, 8, P], dt.bfloat16, tag="wqks")
            wvs = pp.tile([P, 8, 512], dt.bfloat16, tag="wvs")
            wpjs = pp.tile([P, 4, D], dt.bfloat16, tag="wpjs")

            QT = pp.tile([P, 4, S], dt.bfloat16, tag="QT")  # [hd pair, pair, s]
            KT = pp.tile([P, 4, S], dt.bfloat16, tag="KT")
            V2 = pp.tile([P, NHL, NKC, 96], dt.bfloat16, tag="V2")  # [k, h, kc, 64 V | 32 ones]
            nc.gpsimd.memset(V2[:, :, :, 64:96], 1.0)
            yT = pp.tile([P, 4, S], dt.bfloat16, tag="yT")  # [vdim pair, pair, s]

            # stage inputs in first-use order, finely split so the first
            # matmuls start as soon as possible
            nc.sync.dma_start(wqks[:, 0], wqk[:, 0])
            for dcp in range(4):
                nc.sync.dma_start(xT[:, 2 * dcp:2 * dcp + 2, 0:1024],
                                  xt[:, 2 * dcp:2 * dcp + 2, 0:1024])
            nc.sync.dma_start(wqks[:, 4], wqk[:, 4])
            nc.sync.dma_start(xT[:, :, 1024:2048], xt[:, :, 1024:2048])
            for p in range(1, 4):
                nc.sync.dma_start(wqks[:, p], wqk[:, p])
                nc.sync.dma_start(wqks[:, p + 4], wqk[:, p + 4])
            nc.sync.dma_start(wvs[:], wv[:])
            nc.sync.dma_start(wpjs[:], wpj[:])

            # ---- fused QKV production + attention + interleaved projection ----
            # PSUM: psS 2x[P,1024] (4 banks, shared by qkv-production and
            # scores) + psV 2x[P,1024] (4 banks, shared by pv and proj)
            with tc.tile_pool(name="ta", bufs=2) as ta, \
                 tc.tile_pool(name="tpt", bufs=4) as tpt, \
                 tc.tile_pool(name="tp", bufs=4) as tp, \
                 tc.tile_pool(name="psS", bufs=2, space="PSUM") as psS, \
                 tc.tile_pool(name="psV", bufs=2, space="PSUM") as psV:

                def pairQK_half(p, hf):
                    """Q and K for pair p, q-cols hf*1024..+1024."""
                    for ch in (p, p + 4):
                        psq = psS.tile([P, QB], dt.float32, tag="sc")
                        for sb2 in range(2):
                            sb = hf * 2 + sb2
                            for dc in range(8):
                                nc.tensor.matmul(psq[:, sb2 * 512:(sb2 + 1) * 512],
                                                 wqks[:, ch, dc, :],
                                                 xT[:, dc, sb * 512:(sb + 1) * 512],
                                                 start=(dc == 0), stop=(dc == 7))
                        dst = QT if ch < 4 else KT
                        nc.vector.tensor_copy(
                            dst[:, ch % 4, hf * QB:(hf + 1) * QB], psq[:])

                def V_half(hf):
                    """V for all 8 heads, k-chunks hf*8..+8 (2 s-chunks/tile)."""
                    for scp in range(4):
                        psv = psS.tile([P, QB], dt.float32, tag="sc")
                        for sc2 in range(2):
                            sc = hf * 8 + scp * 2 + sc2
                            for dc in range(8):
                                nc.tensor.matmul(psv[:, sc2 * 512:(sc2 + 1) * 512],
                                                 xT[:, dc, sc * P:(sc + 1) * P],
                                                 wvs[:, dc, :],
                                                 start=(dc == 0), stop=(dc == 7))
                        nc.vector.tensor_copy(
                            V2[:, :, hf * 8 + scp * 2:hf * 8 + scp * 2 + 2, 0:64],
                            psv[:].rearrange("p (a b c) -> p b a c", a=2, b=8))

                def attn_block(h, qb):
                    pr = h // 2
                    half = slice(0, 64) if h % 2 == 0 else slice(64, P)
                    nkc = (qb + 1) * 8
                    kcA = qb * 8 + 3        # last k-chunk writing the low q-half
                    # two independent 1-bank pv tiles: the low half's norm chain
                    # runs while PV still accumulates the high half
                    pvA = psV.tile([P, 512], dt.float32, tag="pvA")
                    pvB = psV.tile([P, 512], dt.float32, tag="pvB")
                    pv_h = [pvA, pvB]
                    pend = []  # [(kc, pT tile, qlo)] — PV trails scores by 2

                    def emit_pv(kc, pT_t, qlo):
                        q0 = qlo
                        while q0 < QB:
                            q1 = min((q0 // 512 + 1) * 512, QB)  # stay within one PSUM bank
                            hb = q0 // 512
                            last = kcA if hb == 0 else nkc - 1
                            nc.tensor.matmul(pv_h[hb][0:96, q0 - hb * 512:q1 - hb * 512],
                                             V2[:, h, kc, :], pT_t[:, q0:q1],
                                             start=(kc == 0), stop=(kc == last),
                                             skip_group_check=True)
                            q0 = q1

                    def norm_half(hb):
                        # y = pv * (1/sums); sums dup on rows 64:96
                        pv_ps = pv_h[hb]
                        trc = ta.tile([P, 512], dt.float32, tag="trc")
                        if RECIP_MODE == "dve":
                            nc.vector.reciprocal(trc[64:96, :], pv_ps[64:96, :])
                        elif RECIP_MODE == "copy":
                            nc.vector.tensor_copy(trc[64:96, :], pv_ps[64:96, :])
                        else:  # lnexp on ACT
                            tln = ta.tile([P, 512], dt.float32, tag="tln")
                            nc.scalar.activation(tln[64:96, :], pv_ps[64:96, :], AF.Ln)
                            nc.scalar.activation(trc[64:96, :], tln[64:96, :], AF.Exp,
                                                 scale=-1.0)
                        rsh = ta.tile([64, 512], dt.float32, tag="rsh")
                        nc.sync.dma_start(rsh[0:32, :], trc[64:96, :])
                        nc.sync.dma_start(rsh[32:64, :], trc[64:96, :])
                        span = slice(qb * QB + hb * 512, qb * QB + (hb + 1) * 512)
                        if h % 2 == 0:
                            nc.vector.tensor_tensor(yT[0:64, pr, span],
                                                    pv_ps[0:64, :], rsh[:],
                                                    mybir.AluOpType.mult)
                        else:
                            ytmp = ta.tile([64, 512], dt.bfloat16, tag="ytmp")
                            nc.vector.tensor_tensor(ytmp[:], pv_ps[0:64, :], rsh[:],
                                                    mybir.AluOpType.mult)
                            nc.sync.dma_start(yT[64:P, pr, span], ytmp[:])

                    for kc in range(nkc):
                        qlo = max(0, kc * P - qb * QB)
                        sc_ps = psS.tile([P, QB], dt.float32, tag="sc")
                        q0 = qlo
                        while q0 < QB:
                            q1 = min((q0 // 512 + 1) * 512, QB)  # stay within one PSUM bank
                            nc.tensor.matmul(sc_ps[:, q0:q1],
                                             KT[half, pr, kc * P:(kc + 1) * P],
                                             QT[half, pr, qb * QB + q0:qb * QB + q1],
                                             start=True, stop=True)
                            q0 = q1
                        diag = kc * P >= qb * QB
                        if diag and MASK_MODE == "dve":
                            nc.vector.tensor_tensor(sc_ps[:, qlo:qlo + P],
                                                    sc_ps[:, qlo:qlo + P], maskT[:],
                                                    mybir.AluOpType.add)
                        pT_t = tpt.tile([P, QB], dt.bfloat16, tag="pT")
                        nc.scalar.activation(pT_t[:, qlo:QB], sc_ps[:, qlo:QB], AF.Exp)
                        if diag and MASK_MODE == "pool":  # zero p on k>q corner
                            nc.gpsimd.affine_select(
                                out=pT_t[:, qlo:qlo + P], in_=pT_t[:, qlo:qlo + P],
                                compare_op=mybir.AluOpType.is_ge, fill=0.0,
                                base=0, pattern=[[1, P]], channel_multiplier=-1)
                        pend.append((kc, pT_t, qlo))
                        if len(pend) > 2:
                            e = pend.pop(0)
                            emit_pv(*e)
                            if e[0] == kcA:
                                norm_half(0)
                    for e in pend:
                        emit_pv(*e)
                        if e[0] == kcA:
                            norm_half(0)
                    norm_half(1)

                def proj_chunk(sc):
                    # psum from the scores ring (free once attention is done)
                    pps = psS.tile([P, QB], dt.float32, tag="sc")
                    for oc in range(2):
                        for pc in range(4):
                            nc.tensor.matmul(pps[:, oc * 512:(oc + 1) * 512],
                                             yT[:, pc, sc * P:(sc + 1) * P],
                                             wpjs[:, pc, oc * 512:(oc + 1) * 512],
                                             start=(pc == 0), stop=(pc == 3))
                    so = tp.tile([P, D], dt.float32, tag="so")
                    # alternate copy engine so neither ACT nor DVE throttles PE
                    if sc % 2 == 0:
                        nc.vector.tensor_copy(so[:], pps[:])
                    else:
                        nc.scalar.copy(so[:], pps[:])
                    nc.sync.dma_start(out[sc * P:(sc + 1) * P, :], so[:])

                # solid QKV phase (dense PE stream keeps the p-state ramp warm)
                for p in range(4):
                    pairQK_half(p, 0)
                    pairQK_half(p, 1)
                V_half(0)
                V_half(1)
                if ORDER == "qbmajor":
                    # qb0 first; then qb1 with proj chunks of the finished low
                    # half interleaved as PE fill work in the ACT-bound region
                    for h in range(NHL):
                        attn_block(h, 0)
                    for h in range(NHL):
                        attn_block(h, 1)
                        proj_chunk(h)
                    for sc in range(8, 16):
                        proj_chunk(sc)
                else:
                    for h in range(NHL):
                        attn_block(h, 0)
                        attn_block(h, 1)
                    for sc in range(16):
                        proj_chunk(sc)

    nc.compile()
    return nc


def prepare_inputs(x, Wqkv, Wproj):
    """Pack per-core bf16 inputs. Core c: batch c//2, heads (c%2)*8 .. +8."""
    from ml_dtypes import bfloat16
    x = np.asarray(x, dtype=np.float32)
    Wqkv = np.asarray(Wqkv, dtype=np.float32)
    Wproj = np.asarray(Wproj, dtype=np.float32)
    scale = 1.0 / np.sqrt(HD)
    # Wqkv rows d = dc*128 + p
    Wq = (Wqkv[:, :D] * scale).reshape(8, P, H, HD)  # [dc, p, head, hd]
    Wk = Wqkv[:, D:2 * D].reshape(8, P, H, HD)
    Wv_ = Wqkv[:, 2 * D:].reshape(8, P, H, HD)
    in_maps = []
    for c in range(8):
        b, g = c // 2, c % 2
        hg = g * NHL
        wqk = np.empty((P, 8, 8, P), dtype=np.float32)
        for ch in range(4):
            wqk[:, ch, :, 0:64] = Wq[:, :, hg + 2 * ch, :].transpose(1, 0, 2)
            wqk[:, ch, :, 64:P] = Wq[:, :, hg + 2 * ch + 1, :].transpose(1, 0, 2)
            wqk[:, ch + 4, :, 0:64] = Wk[:, :, hg + 2 * ch, :].transpose(1, 0, 2)
            wqk[:, ch + 4, :, 64:P] = Wk[:, :, hg + 2 * ch + 1, :].transpose(1, 0, 2)
        # wv[p, dc, h*64+j] = Wv[dc*128+p, (hg+h)*64+j]
        wv = Wv_[:, :, hg:hg + NHL, :].reshape(8, P, NHL * HD).transpose(1, 0, 2)
        wpj = np.empty((P, 4, D), dtype=np.float32)
        for pc in range(4):
            wpj[0:64, pc, :] = Wproj[HD * (hg + 2 * pc):HD * (hg + 2 * pc) + HD, :]
            wpj[64:P, pc, :] = Wproj[HD * (hg + 2 * pc + 1):HD * (hg + 2 * pc + 1) + HD, :]
        # xt[p, dc, s] = x[b, s, dc*128+p]
        xt = np.ascontiguousarray(x[b].T.reshape(8, P, S).transpose(1, 0, 2))
        in_maps.append({
            "xt": xt.astype(bfloat16),
            "wqk": np.ascontiguousarray(wqk).astype(bfloat16),
            "wv": np.ascontiguousarray(wv).astype(bfloat16),
            "wpj": wpj.astype(bfloat16),
        })
    return in_maps


def combine_outputs(results):
    out = np.empty((B, S, D), dtype=np.float32)
    for b in range(B):
        out[b] = results[2 * b]["out"] + results[2 * b + 1]["out"]
    return out


_NC_CACHE = None


def get_nc():
    global _NC_CACHE
    if _NC_CACHE is None:
        _NC_CACHE = build_nc()
    return _NC_CACHE


def kernel(x, Wqkv, Wproj):
    from concourse.bass_utils import run_bass_kernel_spmd
    nc = get_nc()
    in_maps = prepare_inputs(x, Wqkv, Wproj)
    res = run_bass_kernel_spmd(nc, in_maps, core_ids=list(range(8)))
    return combine_outputs(res.results)


if __name__ == "__main__":
    rng = np.random.default_rng(0)
    x = rng.standard_normal((B, S, D), dtype=np.float32)
    Wqkv = (rng.standard_normal((D, 3 * D), dtype=np.float32) / np.sqrt(D)).astype(np.float32)
    Wproj = (rng.standard_normal((D, D), dtype=np.float32) / np.sqrt(D)).astype(np.float32)
    y = kernel(x, Wqkv, Wproj)
    print("ok", y.shape, float(np.abs(y).max()))


# revision 45
# speedup vs baseline: 1.2580x; 1.0781x over previous
"""Causal multi-head attention (B=4, S=2048, D=1024, H=16) on 8 Trainium2 NeuronCores.

Sharding: core c handles batch c//2 and head-group c%2 (8 of 16 heads).
Each core computes its 8 heads' qkv projection, causal attention, and its
slice of the output projection; the host sums the two half-head partials
per batch.

All matmul operands are bf16 (host-prepped): x arrives pre-transposed as
xt[d, s], weights in matmul-ready layouts with the 1/sqrt(hd) softmax
scale folded into Wq. V is produced directly in natural [k, vdim] layout
by using x^T chunks as the stationary operand, so no on-chip transposes
are needed. Softmax row-sums ride along the PV matmul as 32 ones-columns
in the stationary operand; the reciprocal runs on DVE.

Self-contained: hardcodes shapes; imports concourse from the container's
trn_rl_repo. kernel(**inputs) takes full inputs, returns full output.
"""
import sys

for _p in ("/opt/trn_rl_repo", "/root/.axon_site/_ro/trn_rl_repo"):
    if _p not in sys.path:
        sys.path.append(_p)

import numpy as np

import concourse.bass as bass
import concourse.mybir as mybir
import concourse.tile as tile
from concourse import bacc

B, S, D, H = 4, 2048, 1024, 16
HD = D // H            # 64
NHL = 8                # heads per core
QB = 1024              # attention q-block
NKC = S // 128         # 16 k-chunks per sequence
dt = mybir.dt
AF = mybir.ActivationFunctionType
P = 128


RECIP_MODE = "dve"  # 'dve' | 'copy' (timing A/B only, wrong values) | 'lnexp'
MASK_MODE = "pool"  # 'pool' (gpsimd zero-fill on pT) | 'dve' (-1e30 add on scores)
ORDER = "paired"    # 'qbmajor' (proj interleaved into qb1) | 'paired'
NORM_BCAST = "dma"  # 'dma' (2 sbuf-sbuf copies) | 'pool' (partition_broadcast:
                    # BROKEN in this toolchain — ucode no-ops, do not use)
DEPTH = 3           # chunks PV trails the scores/exp stream by


def build_nc(repeat=1):
    nc = bacc.Bacc("TRN2", target_bir_lowering=False, debug=False)

    # xt[p, dc, s] = x[s, dc*128 + p]  (pre-transposed, bf16)
    xt = nc.dram_tensor("xt", [P, 8, S], dt.bfloat16, kind="ExternalInput")
    # wqk[p, ch, dc, j]: ch 0-3 q-pairs, 4-7 k-pairs; j = hd of head pair
    wqk = nc.dram_tensor("wqk", [P, 8, 8, P], dt.bfloat16, kind="ExternalInput")
    # wv[p, dc, h*64+j] = Wv[dc*128+p, (hg+h)*64+j]
    wv = nc.dram_tensor("wv", [P, 8, 512], dt.bfloat16, kind="ExternalInput")
    # wpj[j, pc, :]: rows = vdim of head pair pc
    wpj = nc.dram_tensor("wpj", [P, 4, D], dt.bfloat16, kind="ExternalInput")
    out = nc.dram_tensor("out", [S, D], dt.float32, kind="ExternalOutput")

    from contextlib import ExitStack
    with tile.TileContext(nc) as tc, ExitStack() as _rep:
        if repeat > 1:
            _rep.enter_context(tc.For_i(0, repeat, 1))
        with tc.tile_pool(name="persist", bufs=1) as pp:

            if MASK_MODE == "dve":
                # causal mask tile: 0 where f>=p else -1e30
                maskT = pp.tile([P, P], dt.float32, tag="maskT")
                nc.gpsimd.memset(maskT[:], 0.0)
                nc.gpsimd.affine_select(
                    out=maskT[:], in_=maskT[:],
                    compare_op=mybir.AluOpType.is_ge, fill=-1e30,
                    base=0, pattern=[[1, P]], channel_multiplier=-1)

            xT = pp.tile([P, 8, S], dt.bfloat16, tag="xT")
            wqks = pp.tile([P, 8, 8, P], dt.bfloat16, tag="wqks")
            wvs = pp.tile([P, 8, 512], dt.bfloat16, tag="wvs")
            wpjs = pp.tile([P, 4, D], dt.bfloat16, tag="wpjs")

            QT = pp.tile([P, 4, S], dt.bfloat16, tag="QT")  # [hd pair, pair, s]
            KT = pp.tile([P, 4, S], dt.bfloat16, tag="KT")
            V2 = pp.tile([P, NHL, NKC, 96], dt.bfloat16, tag="V2")  # [k, h, kc, 64 V | 32 ones]
            nc.gpsimd.memset(V2[:, :, :, 64:96], 1.0)
            yT = pp.tile([P, 4, S], dt.bfloat16, tag="yT")  # [vdim pair, pair, s]

            # stage inputs in first-use order, finely split so the first
            # matmuls start as soon as possible
            nc.sync.dma_start(wqks[:, 0], wqk[:, 0])
            for dcp in range(4):
                nc.sync.dma_start(xT[:, 2 * dcp:2 * dcp + 2, 0:1024],
                                  xt[:, 2 * dcp:2 * dcp + 2, 0:1024])
            nc.sync.dma_start(wqks[:, 4], wqk[:, 4])
            nc.sync.dma_start(xT[:, :, 1024:2048], xt[:, :, 1024:2048])
            for p in range(1, 4):
                nc.sync.dma_start(wqks[:, p], wqk[:, p])
                nc.sync.dma_start(wqks[:, p + 4], wqk[:, p + 4])
            nc.sync.dma_start(wvs[:], wv[:])
            nc.sync.dma_start(wpjs[:], wpj[:])

            # ---- fused QKV production + attention + interleaved projection ----
            # PSUM: psS 2x[P,1024] (4 banks, shared by qkv-production and
            # scores) + psV 2x[P,1024] (4 banks, shared by pv and proj)
            with tc.tile_pool(name="ta", bufs=2) as ta, \
                 tc.tile_pool(name="tpt", bufs=DEPTH + 2) as tpt, \
                 tc.tile_pool(name="tp", bufs=4) as tp, \
                 tc.tile_pool(name="psS", bufs=2, space="PSUM") as psS, \
                 tc.tile_pool(name="psV", bufs=2, space="PSUM") as psV:

                def pairQK_half(p, hf):
                    """Q and K for pair p, q-cols hf*1024..+1024."""
                    for ch in (p, p + 4):
                        psq = psS.tile([P, QB], dt.float32, tag="sc")
                        for sb2 in range(2):
                            sb = hf * 2 + sb2
                            for dc in range(8):
                                nc.tensor.matmul(psq[:, sb2 * 512:(sb2 + 1) * 512],
                                                 wqks[:, ch, dc, :],
                                                 xT[:, dc, sb * 512:(sb + 1) * 512],
                                                 start=(dc == 0), stop=(dc == 7))
                        dst = QT if ch < 4 else KT
                        nc.vector.tensor_copy(
                            dst[:, ch % 4, hf * QB:(hf + 1) * QB], psq[:])

                def V_half(hf):
                    """V for all 8 heads, k-chunks hf*8..+8 (2 s-chunks/tile)."""
                    for scp in range(4):
                        psv = psS.tile([P, QB], dt.float32, tag="sc")
                        for sc2 in range(2):
                            sc = hf * 8 + scp * 2 + sc2
                            for dc in range(8):
                                nc.tensor.matmul(psv[:, sc2 * 512:(sc2 + 1) * 512],
                                                 xT[:, dc, sc * P:(sc + 1) * P],
                                                 wvs[:, dc, :],
                                                 start=(dc == 0), stop=(dc == 7))
                        nc.vector.tensor_copy(
                            V2[:, :, hf * 8 + scp * 2:hf * 8 + scp * 2 + 2, 0:64],
                            psv[:].rearrange("p (a b c) -> p b a c", a=2, b=8))

                # -- attention as one flat cross-block pipeline: PV trails the
                # scores/exp stream by 2 chunks globally, so the pipeline
                # never drains at block boundaries
                pv_map = {}  # (h, qb) -> [pvA, pvB]

                def scores_exp(h, qb, kc):
                    pr = h // 2
                    half = slice(0, 64) if h % 2 == 0 else slice(64, P)
                    qlo = max(0, kc * P - qb * QB)
                    sc_ps = psS.tile([P, QB], dt.float32, tag="sc")
                    q0 = qlo
                    while q0 < QB:
                        q1 = min((q0 // 512 + 1) * 512, QB)  # stay within one PSUM bank
                        nc.tensor.matmul(sc_ps[:, q0:q1],
                                         KT[half, pr, kc * P:(kc + 1) * P],
                                         QT[half, pr, qb * QB + q0:qb * QB + q1],
                                         start=True, stop=True)
                        q0 = q1
                    diag = kc * P >= qb * QB
                    if diag and MASK_MODE == "dve":
                        nc.vector.tensor_tensor(sc_ps[:, qlo:qlo + P],
                                                sc_ps[:, qlo:qlo + P], maskT[:],
                                                mybir.AluOpType.add)
                    pT_t = tpt.tile([P, QB], dt.bfloat16, tag="pT")
                    nc.scalar.activation(pT_t[:, qlo:QB], sc_ps[:, qlo:QB], AF.Exp)
                    if diag and MASK_MODE == "pool":  # zero p on k>q corner
                        nc.gpsimd.affine_select(
                            out=pT_t[:, qlo:qlo + P], in_=pT_t[:, qlo:qlo + P],
                            compare_op=mybir.AluOpType.is_ge, fill=0.0,
                            base=0, pattern=[[1, P]], channel_multiplier=-1)
                    return (h, qb, kc, pT_t, qlo)

                def norm_half(h, qb, hb):
                    # y = pv * (1/sums); sums dup on rows 64:96
                    pv_ps = pv_map[(h, qb)][hb]
                    trc = ta.tile([P, 512], dt.float32, tag="trc")
                    if RECIP_MODE == "dve":
                        nc.vector.reciprocal(trc[64:96, :], pv_ps[64:96, :])
                    elif RECIP_MODE == "copy":
                        nc.vector.tensor_copy(trc[64:96, :], pv_ps[64:96, :])
                    else:  # lnexp on ACT
                        tln = ta.tile([P, 512], dt.float32, tag="tln")
                        nc.scalar.activation(tln[64:96, :], pv_ps[64:96, :], AF.Ln)
                        nc.scalar.activation(trc[64:96, :], tln[64:96, :], AF.Exp,
                                             scale=-1.0)
                    rsh = ta.tile([64, 512], dt.float32, tag="rsh")
                    if NORM_BCAST == "pool":
                        nc.gpsimd.partition_broadcast(rsh[:], trc[64:65, :])
                    else:
                        nc.sync.dma_start(rsh[0:32, :], trc[64:96, :])
                        nc.sync.dma_start(rsh[32:64, :], trc[64:96, :])
                    span = slice(qb * QB + hb * 512, qb * QB + (hb + 1) * 512)
                    # odd heads write partitions 64:128 directly — DVE lanes
                    # shift across partition bases (verified on hw)
                    rows = slice(0, 64) if h % 2 == 0 else slice(64, P)
                    nc.vector.tensor_tensor(yT[rows, h // 2, span],
                                            pv_ps[0:64, :], rsh[:],
                                            mybir.AluOpType.mult)

                def pop_pv(pend):
                    h, qb, kc, pT_t, qlo = pend.pop(0)
                    nkc = (qb + 1) * 8
                    kcA = qb * 8 + 3      # last k-chunk writing the low q-half
                    if (h, qb) not in pv_map:
                        pvA = psV.tile([P, 512], dt.float32, tag="pvA")
                        pvB = psV.tile([P, 512], dt.float32, tag="pvB")
                        pv_map[(h, qb)] = [pvA, pvB]
                    pv_h = pv_map[(h, qb)]
                    q0 = qlo
                    while q0 < QB:
                        q1 = min((q0 // 512 + 1) * 512, QB)  # stay within one PSUM bank
                        hb = q0 // 512
                        last = kcA if hb == 0 else nkc - 1
                        nc.tensor.matmul(pv_h[hb][0:96, q0 - hb * 512:q1 - hb * 512],
                                         V2[:, h, kc, :], pT_t[:, q0:q1],
                                         start=(kc == 0), stop=(kc == last),
                                         skip_group_check=True)
                        q0 = q1
                    if kc == kcA:
                        norm_half(h, qb, 0)
                    if kc == nkc - 1:
                        norm_half(h, qb, 1)

                def proj_chunk(sc):
                    # psum from the scores ring (free once attention is done)
                    pps = psS.tile([P, QB], dt.float32, tag="sc")
                    for oc in range(2):
                        for pc in range(4):
                            nc.tensor.matmul(pps[:, oc * 512:(oc + 1) * 512],
                                             yT[:, pc, sc * P:(sc + 1) * P],
                                             wpjs[:, pc, oc * 512:(oc + 1) * 512],
                                             start=(pc == 0), stop=(pc == 3))
                    so = tp.tile([P, D], dt.float32, tag="so")
                    # alternate copy engine so neither ACT nor DVE throttles PE
                    if sc % 2 == 0:
                        nc.vector.tensor_copy(so[:], pps[:])
                    else:
                        nc.scalar.copy(so[:], pps[:])
                    nc.sync.dma_start(out[sc * P:(sc + 1) * P, :], so[:])

                # solid QKV phase (dense PE stream keeps the p-state ramp warm)
                for p in range(4):
                    pairQK_half(p, 0)
                    pairQK_half(p, 1)
                V_half(0)
                V_half(1)
                if ORDER == "qbmajor":
                    tasks = [(h, qb, kc) for qb in (0, 1) for h in range(NHL)
                             for kc in range((qb + 1) * 8)]
                else:  # paired
                    tasks = [(h, qb, kc) for h in range(NHL) for qb in (0, 1)
                             for kc in range((qb + 1) * 8)]
                pend = []
                for t in tasks:
                    pend.append(scores_exp(*t))
                    if len(pend) > DEPTH:
                        pop_pv(pend)
                while pend:
                    pop_pv(pend)
                for sc in range(16):
                    proj_chunk(sc)

    nc.compile()
    return nc


def prepare_inputs(x, Wqkv, Wproj):
    """Pack per-core bf16 inputs. Core c: batch c//2, heads (c%2)*8 .. +8."""
    from ml_dtypes import bfloat16
    x = np.asarray(x, dtype=np.float32)
    Wqkv = np.asarray(Wqkv, dtype=np.float32)
    Wproj = np.asarray(Wproj, dtype=np.float32)
    scale = 1.0 / np.sqrt(HD)
    # Wqkv rows d = dc*128 + p
    Wq = (Wqkv[:, :D] * scale).reshape(8, P, H, HD)  # [dc, p, head, hd]
    Wk = Wqkv[:, D:2 * D].reshape(8, P, H, HD)
    Wv_ = Wqkv[:, 2 * D:].reshape(8, P, H, HD)
    in_maps = []
    for c in range(8):
        b, g = c // 2, c % 2
        hg = g * NHL
        wqk = np.empty((P, 8, 8, P), dtype=np.float32)
        for ch in range(4):
            wqk[:, ch, :, 0:64] = Wq[:, :, hg + 2 * ch, :].transpose(1, 0, 2)
            wqk[:, ch, :, 64:P] = Wq[:, :, hg + 2 * ch + 1, :].transpose(1, 0, 2)
            wqk[:, ch + 4, :, 0:64] = Wk[:, :, hg + 2 * ch, :].transpose(1, 0, 2)
            wqk[:, ch + 4, :, 64:P] = Wk[:, :, hg + 2 * ch + 1, :].transpose(1, 0, 2)
        # wv[p, dc, h*64+j] = Wv[dc*128+p, (hg+h)*64+j]
        wv = Wv_[:, :, hg:hg + NHL, :].reshape(8, P, NHL * HD).transpose(1, 0, 2)
        wpj = np.empty((P, 4, D), dtype=np.float32)
        for pc in range(4):
            wpj[0:64, pc, :] = Wproj[HD * (hg + 2 * pc):HD * (hg + 2 * pc) + HD, :]
            wpj[64:P, pc, :] = Wproj[HD * (hg + 2 * pc + 1):HD * (hg + 2 * pc + 1) + HD, :]
        # xt[p, dc, s] = x[b, s, dc*128+p]
        xt = np.ascontiguousarray(x[b].T.reshape(8, P, S).transpose(1, 0, 2))
        in_maps.append({
            "xt": xt.astype(bfloat16),
            "wqk": np.ascontiguousarray(wqk).astype(bfloat16),
            "wv": np.ascontiguousarray(wv).astype(bfloat16),
            "wpj": wpj.astype(bfloat16),
        })
    return in_maps


def combine_outputs(results):
    out = np.empty((B, S, D), dtype=np.float32)
    for b in range(B):
        out[b] = results[2 * b]["out"] + results[2 * b + 1]["out"]
    return out


_NC_CACHE = None


def get_nc():
    global _NC_CACHE
    if _NC_CACHE is None:
        _NC_CACHE = build_nc()
    return _NC_CACHE


def kernel(x, Wqkv, Wproj):
    from concourse.bass_utils import run_bass_kernel_spmd
    nc = get_nc()
    in_maps = prepare_inputs(x, Wqkv, Wproj)
    res = run_bass_kernel_spmd(nc, in_maps, core_ids=list(range(8)))
    return combine_outputs(res.results)


if __name__ == "__main__":
    rng = np.random.default_rng(0)
    x = rng.standard_normal((B, S, D), dtype=np.float32)
    Wqkv = (rng.standard_normal((D, 3 * D), dtype=np.float32) / np.sqrt(D)).astype(np.float32)
    Wproj = (rng.standard_normal((D, D), dtype=np.float32) / np.sqrt(D)).astype(np.float32)
    y = kernel(x, Wqkv, Wproj)
    print("ok", y.shape, float(np.abs(y).max()))
